# revision 17
# baseline (speedup 1.0000x reference)
"""DeepSets segment-reduce kernel for 8x Trainium2 NeuronCores.

Strategy (all shapes hardcoded for N=500000, C=H=128, O=64, NSEG=2048):
  - Transposed activation layout: features on SBUF partitions, nodes on the
    free axis, so segment reductions are free-axis DVE ops.
  - Whole-segment sharding: every segment is assigned entirely to one core,
    round-robin by global sorted-width rank.  All 8 cores then share an
    identical compile-time slot/tile geometry (SPMD-safe); per-core padding
    is <1%.  No collective is needed - the host gather is the unshard.
  - Encoder BN is folded into the linear weights (W' = W * g*rsqrt(v+eps),
    b' = (b-m)*g*rsqrt(v+eps) + beta), so each layer is relu(W'x + b').
  - bf16 everywhere on the streaming path (x, weights, h tiles) for
    1 cycle/row matmuls, FWL weight loads and half the HBM traffic; PSUM
    stays fp32 (TRN2), biases stay fp32 via the ACT bias / TS scalar port.
  - A large negative pad mask is injected into layer-3 PSUM by a rank-1
    matmul (-BIG x is_pad) over the per-slot tail windows.  Pad columns
    fall below zero, so they contribute exactly 0 to post-relu segment
    sums and never win the post-relu segment max (all-pad slots give 0,
    matching the reference's empty-segment zero).
  - Superblock DMA: one xT/aux dma_start covers ~2048 columns (several
    tiles), amortizing the 128-descriptor queue cost on the Sync engine.
  - Chunk-pair layer-major PE schedule: two tiles share one [H, 1024]
    PSUM allocation (2 banks; tile B pinned at column 512 so no matmul
    crosses a bank).  Each layer runs both tiles' matmuls back-to-back
    off one LDWEIGHTS, keeping the PE dense (warm HAM clock) and halving
    weight loads.
  - Engine balance: ACT does relu1+relu2 per tile (PSUM->SBUF, bias via
    the ACT port) plus a tunable share of relu3; DVE does the remaining
    relu3 (tensor_scalar add-bias/max-0 per tile) and per-slot fused
    sum/max accumulation:  tensor_scalar(out=scratch, in0=h3 window,
    accum_out=reduce(out, op1)) reading bf16 SBUF, which qualifies for
    the DVE 2x/4x packed modes; the 1x tensor_reduce path is the
    fallback (KERNEL_SUMMAX=reduce).
  - Final projection out = [sum|max|mean] @ Wo'.T + bo' runs per core on
    its own 256 segments; mean is handled by projecting sums through the
    mean block of Wo' and row-scaling by 1/count.
"""

import os
import sys

import numpy as np

if "/opt/trn_rl_repo" not in sys.path:
    sys.path.insert(0, "/opt/trn_rl_repo")

import ml_dtypes

import concourse.bacc as bacc
import concourse.mybir as mybir
import concourse.tile as tile
from concourse import bass_utils

EPS = 1e-5
NSEG = 2048
NCORES = 8
C = 128
H = 128
O = 64
S = NSEG // NCORES  # segment slots per core (256)
MAX_TILE = 512  # PSUM bank / fp32 matmul-output limit
SB_COLS = 2048  # superblock width: one xT DMA covers several tiles

R3_ACT_MOD = int(os.environ.get("KERNEL_R3ACT", "3"))  # every Nth tile -> ACT
SUMMAX = os.environ.get("KERNEL_SUMMAX", "ts")  # "ts" | "reduce"

BF16 = ml_dtypes.bfloat16

_compiled_cache = {}


def _fold_bn(W, b, g, be, m, v):
    a = g / np.sqrt(v + EPS)
    Wp = W * a[:, None]
    bp = (b - m) * a + be
    return Wp.astype(np.float32), bp.astype(np.float32)


def _plan_tiles(slot_w):
    """Greedy-pack slots (widths descending) into tiles of <=MAX_TILE cols.

    Returns list of (slot_start, n_slots, padded_width, col_start) and the
    total padded column count.
    """
    tiles = []
    col = 0
    k = 0
    n = len(slot_w)
    while k < n:
        wt = (int(slot_w[k]) + 1) & ~1  # even widths keep windows 4B-aligned
        assert 0 < wt <= MAX_TILE, f"slot width {wt} unsupported"
        d = min(MAX_TILE // wt, n - k)
        tiles.append((k, d, wt, col))
        col += d * wt
        k += d
    return tiles, col


def _build_program(tiles, cols, BIG_DEVICE):
    """Emit the Bass/Tile program shared by all 8 cores."""
    nc = bacc.Bacc(
        "TRN2",
        target_bir_lowering=False,
        debug=False,
        num_devices=NCORES,
    )
    f32 = mybir.dt.float32
    bf16 = mybir.dt.bfloat16

    xT = nc.dram_tensor("xT", [C, cols], bf16, kind="ExternalInput").ap()
    aux = nc.dram_tensor("aux", [1, cols], bf16, kind="ExternalInput").ap()
    w1 = nc.dram_tensor("w1", [C, H], bf16, kind="ExternalInput").ap()
    w2 = nc.dram_tensor("w2", [H, H], bf16, kind="ExternalInput").ap()
    w3 = nc.dram_tensor("w3", [H, H], bf16, kind="ExternalInput").ap()
    b1 = nc.dram_tensor("b1", [H, 1], f32, kind="ExternalInput").ap()
    b3 = nc.dram_tensor("b3", [H, 1], f32, kind="ExternalInput").ap()
    nbig = nc.dram_tensor("nbig", [1, H], bf16, kind="ExternalInput").ap()
    b2 = nc.dram_tensor("b2", [H, 1], f32, kind="ExternalInput").ap()
    wsum = nc.dram_tensor("wsum", [H, O], f32, kind="ExternalInput").ap()
    wmax = nc.dram_tensor("wmax", [H, O], f32, kind="ExternalInput").ap()
    wmean = nc.dram_tensor("wmean", [H, O], f32, kind="ExternalInput").ap()
    bo = nc.dram_tensor("bo", [1, O], f32, kind="ExternalInput").ap()
    # column ch holds the reciprocals for segment chunk ch (128 slots each)
    recip = nc.dram_tensor("recip", [H, S // H], f32, kind="ExternalInput").ap()
    out = nc.dram_tensor("out", [S, O], f32, kind="ExternalOutput").ap()

    # group consecutive tiles into superblocks sharing one xT/aux DMA
    sblocks = []
    cur, cur_col0, cur_cols = [], 0, 0
    for t in tiles:
        _k0, _d, _wt, _col0, _tailw = t
        _tcols = _d * _wt
        if cur and (_col0 + _tcols - cur_col0) > SB_COLS:
            sblocks.append((cur_col0, cur_cols, cur))
            cur = []
        if not cur:
            cur_col0 = _col0
        cur.append(t)
        cur_cols = _col0 + _tcols - cur_col0
    if cur:
        sblocks.append((cur_col0, cur_cols, cur))

    relu = mybir.ActivationFunctionType.Relu
    add = mybir.AluOpType.add
    amax = mybir.AluOpType.max

    with tile.TileContext(nc) as tc:
        with (
            tc.tile_pool(name="const", bufs=1) as cpool,
            tc.tile_pool(name="xin", bufs=3) as xpool,
            tc.tile_pool(name="auxin", bufs=3) as apool,
            tc.tile_pool(name="h1", bufs=4) as h1pool,
            tc.tile_pool(name="h2", bufs=4) as h2pool,
            tc.tile_pool(name="h3", bufs=4) as h3pool,
            tc.tile_pool(name="scr", bufs=2) as scrpool,
            tc.tile_pool(name="acc", bufs=1) as accpool,
            tc.tile_pool(name="ps1", bufs=1, space="PSUM") as ps1,
            tc.tile_pool(name="ps2", bufs=1, space="PSUM") as ps2,
            tc.tile_pool(name="ps3", bufs=1, space="PSUM") as ps3,
            tc.tile_pool(name="pso", bufs=1, space="PSUM") as pso,
        ):
            w1s = cpool.tile([C, H], bf16, tag="w1")
            w2s = cpool.tile([H, H], bf16, tag="w2")
            w3s = cpool.tile([H, H], bf16, tag="w3")
            b1s = cpool.tile([H, 1], f32, tag="b1")
            b3s = cpool.tile([H, 1], f32, tag="b3")
            negbig = cpool.tile([1, H], bf16, tag="negbig")
            b2s = cpool.tile([H, 1], f32, tag="b2")
            wsums = cpool.tile([H, O], f32, tag="wsum")
            wmaxs = cpool.tile([H, O], f32, tag="wmax")
            wmeans = cpool.tile([H, O], f32, tag="wmean")
            bos = cpool.tile([1, O], f32, tag="bo")
            recs = cpool.tile([H, S // H], f32, tag="recip")
            ones = cpool.tile([1, H], f32, tag="ones")

            nc.sync.dma_start(w1s[:], w1)
            nc.sync.dma_start(w2s[:], w2)
            nc.sync.dma_start(w3s[:], w3)
            nc.sync.dma_start(b1s[:], b1)
            nc.sync.dma_start(b3s[:], b3)
            nc.sync.dma_start(b2s[:], b2)
            nc.sync.dma_start(wsums[:], wsum)
            nc.sync.dma_start(wmaxs[:], wmax)
            nc.sync.dma_start(wmeans[:], wmean)
            nc.sync.dma_start(bos[:], bo)
            nc.sync.dma_start(recs[:], recip)
            nc.vector.memset(ones[:], 1.0)
            nc.sync.dma_start(negbig[:], nbig)

            # Persistent per-slot partials (both post-relu, bias included).
            sumP = accpool.tile([H, S], f32, tag="sumP")
            maxP = accpool.tile([H, S], f32, tag="maxP")

            ti = 0  # global tile index (for the relu3 engine split)
            for sb_col0, sb_cols, sbtiles in sblocks:
                xt = xpool.tile([C, SB_COLS], bf16, tag="xt")
                nc.sync.dma_start(
                    xt[:, :sb_cols], xT[:, sb_col0 : sb_col0 + sb_cols]
                )
                need_aux = any(t[4] > 0 for t in sbtiles)
                if need_aux:
                    at = apool.tile([1, SB_COLS], bf16, tag="at")
                    nc.sync.dma_start(
                        at[:, :sb_cols], aux[:, sb_col0 : sb_col0 + sb_cols]
                    )

                for ci in range(0, len(sbtiles), 2):
                    pair = sbtiles[ci : ci + 2]
                    offs = [0, MAX_TILE][: len(pair)]

                    p1 = ps1.tile([H, 2 * MAX_TILE], f32, tag="p1")
                    p2 = ps2.tile([H, 2 * MAX_TILE], f32, tag="p2")
                    p3 = ps3.tile([H, 2 * MAX_TILE], f32, tag="p3")

                    # ---- layer 1: both tiles off one w1 load ----
                    for (k0, d, wt, col0, tailw), off in zip(pair, offs):
                        tcols = d * wt
                        base = col0 - sb_col0
                        nc.tensor.matmul(
                            p1[:, off : off + tcols], w1s[:],
                            xt[:, base : base + tcols],
                        )
                    h1t = []
                    for (k0, d, wt, col0, tailw), off in zip(pair, offs):
                        tcols = d * wt
                        h1 = h1pool.tile([H, MAX_TILE], bf16, tag="h1")
                        nc.scalar.activation(
                            h1[:, :tcols], p1[:, off : off + tcols], relu,
                            bias=b1s[:],
                        )
                        h1t.append(h1)

                    # ---- layer 2 ----
                    for (t, off), h1 in zip(zip(pair, offs), h1t):
                        tcols = t[1] * t[2]
                        nc.tensor.matmul(
                            p2[:, off : off + tcols], w2s[:], h1[:, :tcols]
                        )
                    h2t = []
                    for (k0, d, wt, col0, tailw), off in zip(pair, offs):
                        tcols = d * wt
                        h2 = h2pool.tile([H, MAX_TILE], bf16, tag="h2")
                        nc.scalar.activation(
                            h2[:, :tcols], p2[:, off : off + tcols], relu,
                            bias=b2s[:],
                        )
                        h2t.append(h2)

                    # ---- layer 3 (+ -BIG pad masks) ----
                    for (t, off), h2 in zip(zip(pair, offs), h2t):
                        k0, d, wt, col0, tailw = t
                        tcols = d * wt
                        nc.tensor.matmul(
                            p3[:, off : off + tcols], w3s[:], h2[:, :tcols],
                            start=True, stop=(tailw == 0),
                        )
                    for (k0, d, wt, col0, tailw), off in zip(pair, offs):
                        if tailw <= 0:
                            continue
                        tcols = d * wt
                        base = col0 - sb_col0
                        p3v = p3[:, off : off + tcols].rearrange(
                            "p (d w) -> p d w", d=d
                        )
                        atv = at[:, base : base + tcols].rearrange(
                            "p (d w) -> p d w", d=d
                        )
                        nc.tensor.matmul(
                            p3v[:, :, wt - tailw : wt],
                            negbig[:],
                            atv[:, :, wt - tailw : wt],
                            start=False, stop=True,
                        )

                    # ---- relu3 -> h3 (true post-relu, bias included) ----
                    for (k0, d, wt, col0, tailw), off in zip(pair, offs):
                        tcols = d * wt
                        h3 = h3pool.tile([H, MAX_TILE], bf16, tag="h3")
                        if ti % R3_ACT_MOD == 0:
                            nc.scalar.activation(
                                h3[:, :tcols], p3[:, off : off + tcols],
                                relu, bias=b3s[:],
                            )
                        else:
                            nc.vector.tensor_scalar(
                                h3[:, :tcols], p3[:, off : off + tcols],
                                b3s[:], 0.0, op0=add, op1=amax,
                            )
                        ti += 1

                        # ---- per-slot sum/max accumulation from h3 ----
                        h3v = h3[:, :tcols].rearrange("p (d w) -> p d w", d=d)
                        if SUMMAX == "ts":
                            scr = scrpool.tile([H, MAX_TILE], bf16, tag="scr")
                            for j in range(d):
                                c0 = j * wt
                                k = k0 + j
                                nc.vector.tensor_scalar(
                                    scr[:, c0 : c0 + wt], h3[:, c0 : c0 + wt],
                                    0.0, 0.0, op0=add, op1=add,
                                    accum_out=sumP[:, k : k + 1],
                                )
                                nc.vector.tensor_scalar(
                                    scr[:, c0 : c0 + wt], h3[:, c0 : c0 + wt],
                                    0.0, 0.0, op0=add, op1=amax,
                                    accum_out=maxP[:, k : k + 1],
                                )
                        else:
                            nc.vector.reduce_sum(
                                sumP[:, k0 : k0 + d], h3v,
                                axis=mybir.AxisListType.X,
                            )
                            nc.vector.reduce_max(
                                maxP[:, k0 : k0 + d], h3v,
                                axis=mybir.AxisListType.X,
                            )

            # ---- epilogue: out[k, :] = sum_k @ Wsum + max_k @ Wmax
            #                + (sum_k * recip_k) @ Wmean + bo ----
            for ch in range(S // H):  # 2 chunks of 128 segments
                sl = slice(ch * H, (ch + 1) * H)
                po = pso.tile([H, O], f32, tag="po")
                nc.tensor.matmul(po[:], sumP[:, sl], wsums[:], start=True, stop=False)
                nc.tensor.matmul(po[:], maxP[:, sl], wmaxs[:], start=False, stop=False)
                nc.tensor.matmul(po[:], ones[:], bos[:], start=False, stop=True)

                pm = pso.tile([H, O], f32, tag="pm")
                nc.tensor.matmul(pm[:], sumP[:, sl], wmeans[:], start=True, stop=True)

                om = h1pool.tile([H, O], f32, tag="om")
                nc.vector.tensor_scalar_mul(om[:], pm[:], recs[:, ch : ch + 1])
                ot = h2pool.tile([H, O], f32, tag="ot")
                nc.vector.tensor_tensor(ot[:], po[:], om[:], op=mybir.AluOpType.add)
                nc.sync.dma_start(out[sl, :], ot[:])

    nc.compile()
    return nc


def kernel(**inputs):
    x = np.ascontiguousarray(np.asarray(inputs["x"], dtype=np.float32))
    batch = np.asarray(inputs["batch"]).astype(np.int64)

    # ---- fold BN into the linears ----
    W1p, b1p = _fold_bn(
        np.asarray(inputs["W1"]), np.asarray(inputs["b1"]),
        np.asarray(inputs["g1"]), np.asarray(inputs["be1"]),
        np.asarray(inputs["m1"]), np.asarray(inputs["v1"]),
    )
    W2p, b2p = _fold_bn(
        np.asarray(inputs["W2"]), np.asarray(inputs["b2"]),
        np.asarray(inputs["g2"]), np.asarray(inputs["be2"]),
        np.asarray(inputs["m2"]), np.asarray(inputs["v2"]),
    )
    W3p, b3p = _fold_bn(
        np.asarray(inputs["W3"]), np.asarray(inputs["b3"]),
        np.asarray(inputs["g3"]), np.asarray(inputs["be3"]),
        np.asarray(inputs["m3"]), np.asarray(inputs["v3"]),
    )
    Wop, bop = _fold_bn(
        np.asarray(inputs["Wo"]), np.asarray(inputs["bo"]),
        np.asarray(inputs["go"]), np.asarray(inputs["beo"]),
        np.asarray(inputs["mo"]), np.asarray(inputs["vo"]),
    )

    # bf16 copies of the streaming-path weights (also used for the BIG bound)
    W2b = W2p.astype(BF16).astype(np.float32)
    W3b = W3p.astype(BF16).astype(np.float32)

    # Pad columns are zero in x, so h2_pad is a known constant; BIG pushes the
    # padded layer-3 pre-activation strictly below zero (with margin for the
    # bf16 rounding of x/h tiles).
    h1_pad = np.maximum(b1p, 0.0)
    h2_pad = np.maximum(W2b @ h1_pad + b2p, 0.0)
    v3 = W3b @ h2_pad + b3p
    BIG = float(max(0.0, v3.max()) + 1000.0)

    # ---- whole-segment sharding by sorted-width round-robin rank ----
    counts = np.bincount(batch, minlength=NSEG).astype(np.int64)
    assert np.all(batch[:-1] <= batch[1:]), "batch must be sorted"
    order = np.argsort(-counts, kind="stable")  # segment ids, width desc
    slot_w = np.maximum(counts[order[::NCORES][:S]], 1)  # width of rank 8k
    tiles0, cols = _plan_tiles(slot_w)

    # per-tile tail-window width: the trailing columns of each slot that can
    # be padding on ANY core (only these need the -BIG mask matmul)
    wmat = counts[order[: S * NCORES]].reshape(S, NCORES)  # slot x core widths
    tiles = []
    for k0, d, wt, col0 in tiles0:
        minw = int(wmat[k0 : k0 + d].min())
        tailw = min(wt, (wt - minw + 1) & ~1)
        tiles.append((k0, d, wt, col0, tailw))

    key = (cols, float(BIG), tuple(slot_w.tolist()),
           tuple(t[4] for t in tiles), R3_ACT_MOD, SUMMAX)
    if key not in _compiled_cache:
        _compiled_cache[key] = _build_program(tiles, cols, BIG)
    nc = _compiled_cache[key]

    # column start of each slot
    slot_col = np.zeros(S, dtype=np.int64)
    for k0, d, wt, col0, tailw in tiles:
        for j in range(d):
            slot_col[k0 + j] = col0 + j * wt

    starts = np.searchsorted(batch, np.arange(NSEG), side="left")
    ends = np.searchsorted(batch, np.arange(NSEG), side="right")

    in_maps = []
    for c in range(NCORES):
        segs = order[np.arange(S) * NCORES + c]  # this core's segment ids
        src = np.full(cols, -1, dtype=np.int64)
        for k in range(S):
            s = segs[k]
            cnt = int(counts[s])
            if cnt:
                src[slot_col[k] : slot_col[k] + cnt] = np.arange(
                    starts[s], ends[s]
                )
        real = src >= 0
        xTc = np.zeros((C, cols), dtype=BF16)
        xTc[:, real] = x[src[real]].T.astype(BF16)
        auxc = np.zeros((1, cols), dtype=BF16)
        auxc[0, ~real] = 1.0
        recipc = (1.0 / np.maximum(counts[segs], 1.0)).astype(np.float32)
        in_maps.append(
            dict(
                xT=xTc,
                aux=auxc,
                w1=np.ascontiguousarray(W1p.T.astype(BF16)),
                w2=np.ascontiguousarray(W2p.T.astype(BF16)),
                w3=np.ascontiguousarray(W3p.T.astype(BF16)),
                b1=np.ascontiguousarray(b1p[:, None]),
                b2=np.ascontiguousarray(b2p[:, None]),
                b3=np.ascontiguousarray(b3p[:, None]),
                nbig=np.full((1, H), -BIG, BF16),
                wsum=np.ascontiguousarray(Wop[:, 0:H].T),
                wmax=np.ascontiguousarray(Wop[:, H : 2 * H].T),
                wmean=np.ascontiguousarray(Wop[:, 2 * H : 3 * H].T),
                bo=np.ascontiguousarray(bop[None, :]),
                recip=np.ascontiguousarray(recipc.reshape(S // H, H).T),
            )
        )

    ncores_run = int(os.environ.get("KERNEL_NCORES", str(NCORES)))
    res = bass_utils.run_bass_kernel_spmd(
        nc,
        in_maps[:ncores_run],
        core_ids=list(range(ncores_run)),
        trace=bool(int(os.environ.get("KERNEL_TRACE", "0"))),
        tmpdir=os.environ.get("KERNEL_TRACE_DIR") or None,
    )
    kernel.last_results = res

    out_full = np.zeros((NSEG, O), dtype=np.float32)
    ranks = np.arange(S)
    for c in range(ncores_run):
        out_full[order[ranks * NCORES + c]] = res.results[c]["out"]
    return out_full


# revision 27
# speedup vs baseline: 1.1922x; 1.1922x over previous
"""DeepSets segment-reduce kernel for 8x Trainium2 NeuronCores.

Strategy (all shapes hardcoded for N=500000, C=H=128, O=64, NSEG=2048):
  - Transposed activation layout: features on SBUF partitions, nodes on the
    free axis, so segment reductions are free-axis DVE ops.
  - Whole-segment sharding: every segment is assigned entirely to one core,
    round-robin by global sorted-width rank.  All 8 cores then share an
    identical compile-time slot/tile geometry (SPMD-safe); per-core padding
    is <1%.  No collective is needed - the host gather is the unshard.
  - Encoder BN is folded into the linear weights (W' = W * g*rsqrt(v+eps),
    b' = (b-m)*g*rsqrt(v+eps) + beta), so each layer is relu(W'x + b').
  - bf16 everywhere on the streaming path (x, weights, h tiles) for
    1 cycle/row matmuls, FWL weight loads and half the HBM traffic; PSUM
    stays fp32 (TRN2), biases stay fp32 via the ACT bias / TS scalar port.
  - A large negative pad mask is injected into layer-3 PSUM by a rank-1
    matmul (-BIG x is_pad) over the per-slot tail windows.  Pad columns
    fall below zero, so they contribute exactly 0 to post-relu segment
    sums and never win the post-relu segment max.
  - Engine balance: ACT does relu1+relu2 (PSUM->SBUF, bias via ACT port).
    DVE does a fused per-slot tensor_scalar for layer 3 using the shift
    identity  g3 = max(z3, -b3) = relu(z3+b3) - b3:  out=g3 (bf16 SBUF)
    and accum_out = sum(g3 window)  (TS semantics: out = in0 op0 s1,
    accum_out = reduce(out, op1)).  One reduce_max per tile over g3 gives
    the shifted segment max.  The shifts are exact compile-time
    constants, fixed in the epilogue:  sum += wt*b3  (wt = padded slot
    width; every pad/empty column contributes exactly -b3) and
    max += b3 (all-pad slots come out at -b3, i.e. 0 after the shift,
    matching the reference's empty-segment zero).
  - Final projection out = [sum|max|mean] @ Wo'.T + bo' runs per core on
    its own 256 segments; mean is handled by projecting sums through the
    mean block of Wo' and row-scaling by 1/count.
"""

import os
import sys

import numpy as np

if "/opt/trn_rl_repo" not in sys.path:
    sys.path.insert(0, "/opt/trn_rl_repo")

import ml_dtypes

import concourse.bacc as bacc
import concourse.mybir as mybir
import concourse.tile as tile
from concourse import bass_utils

EPS = 1e-5
NSEG = 2048
NCORES = 8
C = 128
H = 128
O = 64
S = NSEG // NCORES  # segment slots per core (256)
MAX_TILE = 512  # PSUM bank / moving-operand limit
SB_COLS = 2048  # superblock width: one xT DMA covers several tiles
# GPSIMD cannot run TENSOR_TENSOR on TRN2 (no such opcode on Pool) — keep 0.
MAX_GPSIMD = int(os.environ.get("KERNEL_MAX_GPSIMD", "0"))
# Number of slots whose fused relu3+sum runs on ACT (activation accum_out,
# true relu) instead of DVE (shift-trick tensor_scalar) — load balancing.
ACT_SLOTS = int(os.environ.get("KERNEL_ACT_SLOTS", "36"))

BF16 = ml_dtypes.bfloat16

_compiled_cache = {}


def _fold_bn(W, b, g, be, m, v):
    a = g / np.sqrt(v + EPS)
    Wp = W * a[:, None]
    bp = (b - m) * a + be
    return Wp.astype(np.float32), bp.astype(np.float32)


def _plan_tiles(slot_w):
    """Greedy-pack slots (widths descending) into tiles of <=MAX_TILE cols.

    Returns list of (slot_start, n_slots, padded_width, col_start) and the
    total padded column count.
    """
    tiles = []
    col = 0
    k = 0
    n = len(slot_w)
    while k < n:
        wt = (int(slot_w[k]) + 1) & ~1  # even widths keep windows 4B-aligned
        assert 0 < wt <= MAX_TILE, f"slot width {wt} unsupported"
        d = min(MAX_TILE // wt, n - k)
        tiles.append((k, d, wt, col))
        col += d * wt
        k += d
    return tiles, col


def _build_program(tiles, cols, BIG_DEVICE):
    """Emit the Bass/Tile program shared by all 8 cores."""
    nc = bacc.Bacc(
        "TRN2",
        target_bir_lowering=False,
        debug=False,
        num_devices=NCORES,
    )
    f32 = mybir.dt.float32
    bf16 = mybir.dt.bfloat16

    xT = nc.dram_tensor("xT", [C, cols], bf16, kind="ExternalInput").ap()
    aux = nc.dram_tensor("aux", [1, cols], bf16, kind="ExternalInput").ap()
    w1 = nc.dram_tensor("w1", [C, H], bf16, kind="ExternalInput").ap()
    w2 = nc.dram_tensor("w2", [H, H], bf16, kind="ExternalInput").ap()
    w3 = nc.dram_tensor("w3", [H, H], bf16, kind="ExternalInput").ap()
    b1 = nc.dram_tensor("b1", [H, 1], f32, kind="ExternalInput").ap()
    b3 = nc.dram_tensor("b3", [H, 1], f32, kind="ExternalInput").ap()
    nbig = nc.dram_tensor("nbig", [1, H], bf16, kind="ExternalInput").ap()
    b2 = nc.dram_tensor("b2", [H, 1], f32, kind="ExternalInput").ap()
    wsum = nc.dram_tensor("wsum", [H, O], f32, kind="ExternalInput").ap()
    wmax = nc.dram_tensor("wmax", [H, O], f32, kind="ExternalInput").ap()
    wmean = nc.dram_tensor("wmean", [H, O], f32, kind="ExternalInput").ap()
    bo = nc.dram_tensor("bo", [1, O], f32, kind="ExternalInput").ap()
    # column ch holds the reciprocals for segment chunk ch (128 slots each)
    recip = nc.dram_tensor("recip", [H, S // H], f32, kind="ExternalInput").ap()
    nb3 = nc.dram_tensor("nb3", [H, 1], f32, kind="ExternalInput").ap()
    wtb3 = nc.dram_tensor("wtb3", [H, S], f32, kind="ExternalInput").ap()
    mb3 = nc.dram_tensor("mb3", [H, S], f32, kind="ExternalInput").ap()
    out = nc.dram_tensor("out", [S, O], f32, kind="ExternalOutput").ap()

    # group consecutive tiles into superblocks sharing one xT/aux DMA
    sblocks = []
    cur, cur_col0, cur_cols = [], 0, 0
    for t in tiles:
        _k0, _d, _wt, _col0, _tailw = t
        _tcols = _d * _wt
        if cur and (_col0 + _tcols - cur_col0) > SB_COLS:
            sblocks.append((cur_col0, cur_cols, cur))
            cur = []
        if not cur:
            cur_col0 = _col0
        cur.append(t)
        cur_cols = _col0 + _tcols - cur_col0
    if cur:
        sblocks.append((cur_col0, cur_cols, cur))

    with tile.TileContext(nc) as tc:
        with (
            tc.tile_pool(name="const", bufs=1) as cpool,
            tc.tile_pool(name="xin", bufs=3) as xpool,
            tc.tile_pool(name="auxin", bufs=3) as apool,
            tc.tile_pool(name="h1", bufs=4) as h1pool,
            tc.tile_pool(name="h2", bufs=4) as h2pool,
            tc.tile_pool(name="h3", bufs=4) as h3pool,
            tc.tile_pool(name="gh", bufs=4) as ghpool,
            tc.tile_pool(name="acc", bufs=1) as accpool,
            tc.tile_pool(name="ps1", bufs=2, space="PSUM") as ps1,
            tc.tile_pool(name="ps2", bufs=2, space="PSUM") as ps2,
            tc.tile_pool(name="ps3", bufs=2, space="PSUM") as ps3,
            tc.tile_pool(name="pso", bufs=1, space="PSUM") as pso,
        ):
            w1s = cpool.tile([C, H], bf16, tag="w1")
            w2s = cpool.tile([H, H], bf16, tag="w2")
            w3s = cpool.tile([H, H], bf16, tag="w3")
            b1s = cpool.tile([H, 1], f32, tag="b1")
            b3s = cpool.tile([H, 1], f32, tag="b3")
            negbig = cpool.tile([1, H], bf16, tag="negbig")
            b2s = cpool.tile([H, 1], f32, tag="b2")
            wsums = cpool.tile([H, O], f32, tag="wsum")
            wmaxs = cpool.tile([H, O], f32, tag="wmax")
            wmeans = cpool.tile([H, O], f32, tag="wmean")
            bos = cpool.tile([1, O], f32, tag="bo")
            recs = cpool.tile([H, S // H], f32, tag="recip")
            ones = cpool.tile([1, H], f32, tag="ones")
            nb3s = cpool.tile([H, 1], f32, tag="nb3")
            wtb3s = cpool.tile([H, S], f32, tag="wtb3")
            mb3s = cpool.tile([H, S], f32, tag="mb3")

            nc.sync.dma_start(w1s[:], w1)
            nc.sync.dma_start(w2s[:], w2)
            nc.sync.dma_start(w3s[:], w3)
            nc.sync.dma_start(b1s[:], b1)
            nc.sync.dma_start(b3s[:], b3)
            nc.sync.dma_start(b2s[:], b2)
            nc.sync.dma_start(wsums[:], wsum)
            nc.sync.dma_start(wmaxs[:], wmax)
            nc.sync.dma_start(wmeans[:], wmean)
            nc.sync.dma_start(bos[:], bo)
            nc.sync.dma_start(recs[:], recip)
            nc.vector.memset(ones[:], 1.0)
            nc.sync.dma_start(negbig[:], nbig)
            nc.sync.dma_start(nb3s[:], nb3)
            nc.sync.dma_start(wtb3s[:], wtb3)
            nc.sync.dma_start(mb3s[:], mb3)

            # Persistent per-slot partials (both post-relu, bias included).
            sumP = accpool.tile([H, S], f32, tag="sumP")
            maxP = accpool.tile([H, S], f32, tag="maxP")

            relu = mybir.ActivationFunctionType.Relu
            add = mybir.AluOpType.add
            amax = mybir.AluOpType.max

            for sb_col0, sb_cols, sbtiles in sblocks:
                xt = xpool.tile([C, SB_COLS], bf16, tag="xt")
                nc.sync.dma_start(
                    xt[:, :sb_cols], xT[:, sb_col0 : sb_col0 + sb_cols]
                )
                need_aux = any(t[4] > 0 for t in sbtiles)
                if need_aux:
                    at = apool.tile([1, SB_COLS], bf16, tag="at")
                    nc.sync.dma_start(
                        at[:, :sb_cols], aux[:, sb_col0 : sb_col0 + sb_cols]
                    )

                for k0, d, wt, col0, tailw in sbtiles:
                    tcols = d * wt
                    base = col0 - sb_col0
                    xts = xt[:, base : base + tcols]

                    p1 = ps1.tile([H, MAX_TILE], f32, tag="p1")
                    nc.tensor.matmul(p1[:, :tcols], w1s[:], xts)
                    h1 = h1pool.tile([H, MAX_TILE], bf16, tag="h1")
                    nc.scalar.activation(
                        h1[:, :tcols], p1[:, :tcols], relu, bias=b1s[:]
                    )

                    p2 = ps2.tile([H, MAX_TILE], f32, tag="p2")
                    nc.tensor.matmul(p2[:, :tcols], w2s[:], h1[:, :tcols])
                    h2 = h2pool.tile([H, MAX_TILE], bf16, tag="h2")
                    nc.scalar.activation(
                        h2[:, :tcols], p2[:, :tcols], relu, bias=b2s[:]
                    )

                    p3 = ps3.tile([H, MAX_TILE], f32, tag="p3")
                    p3v = p3[:, :tcols].rearrange("p (d w) -> p d w", d=d)
                    if tailw > 0:
                        nc.tensor.matmul(
                            p3[:, :tcols], w3s[:], h2[:, :tcols],
                            start=True, stop=False,
                        )
                        # -BIG into the per-slot tail windows (the only
                        # columns that can be pads on any core).
                        atv = at[:, base : base + tcols].rearrange(
                            "p (d w) -> p d w", d=d
                        )
                        nc.tensor.matmul(
                            p3v[:, :, wt - tailw : wt],
                            negbig[:],
                            atv[:, :, wt - tailw : wt],
                            start=False,
                            stop=True,
                        )
                    else:
                        nc.tensor.matmul(
                            p3[:, :tcols], w3s[:], h2[:, :tcols],
                            start=True, stop=True,
                        )

                    g3 = h3pool.tile([H, MAX_TILE], bf16, tag="g3")
                    # Fused per-slot layer-3 relu + segment sum.  Slot
                    # k < ACT_SLOTS runs on ACT (true relu+bias, accum =
                    # true sum, no correction); the rest run on DVE with
                    # the shift identity g3 = max(z3,-b3) = relu(z3+b3)-b3
                    # (corrected by +wt*b3 / +b3 in the epilogue).
                    for j in range(d):
                        c0 = j * wt
                        k = k0 + j
                        if k < ACT_SLOTS:
                            nc.scalar.activation(
                                g3[:, c0 : c0 + wt],
                                p3[:, c0 : c0 + wt],
                                relu,
                                bias=b3s[:],
                                accum_out=sumP[:, k : k + 1],
                            )
                        else:
                            nc.vector.tensor_scalar(
                                g3[:, c0 : c0 + wt],
                                p3[:, c0 : c0 + wt],
                                nb3s[:],
                                0.0,
                                op0=amax,
                                op1=add,
                                accum_out=sumP[:, k : k + 1],
                            )
                    g3v = g3[:, :tcols].rearrange("p (d w) -> p d w", d=d)
                    if MAX_GPSIMD and wt >= 4:
                        # first max-halving level on the otherwise-idle
                        # GPSIMD, then the 1x DVE reduce on half the data
                        wt2 = wt // 2
                        gh = ghpool.tile([H, MAX_TILE // 2], bf16, tag="gh")
                        ghv = gh[:, : d * wt2].rearrange(
                            "p (d w) -> p d w", d=d
                        )
                        nc.gpsimd.tensor_tensor(
                            ghv, g3v[:, :, 0:wt2], g3v[:, :, wt2 : 2 * wt2],
                            op=amax,
                        )
                        if wt % 2:
                            # odd leftover column folds in via the reduce
                            # over [gh | last col]; simpler: fold leftover
                            # into gh's first column on DVE
                            nc.vector.tensor_tensor(
                                ghv[:, :, 0:1], ghv[:, :, 0:1],
                                g3v[:, :, wt - 1 : wt], op=amax,
                            )
                        nc.vector.reduce_max(
                            maxP[:, k0 : k0 + d], ghv, axis=mybir.AxisListType.X
                        )
                    else:
                        nc.vector.reduce_max(
                            maxP[:, k0 : k0 + d], g3v, axis=mybir.AxisListType.X
                        )

            # ---- epilogue: undo the b3 shift, then project ----
            # sumC = sumP + wt*b3 (true post-relu sums); maxC = maxP + b3
            # (true post-relu maxes; all-pad slots -> exactly 0).
            sumC = accpool.tile([H, S], f32, tag="sumC")
            maxC = accpool.tile([H, S], f32, tag="maxC")
            nc.vector.tensor_tensor(sumC[:], sumP[:], wtb3s[:], op=add)
            nc.vector.tensor_tensor(maxC[:], maxP[:], mb3s[:], op=add)

            # out[k, :] = sum_k @ Wsum + max_k @ Wmax
            #             + (sum_k * recip_k) @ Wmean + bo
            for ch in range(S // H):  # 2 chunks of 128 segments
                sl = slice(ch * H, (ch + 1) * H)
                po = pso.tile([H, O], f32, tag="po")
                nc.tensor.matmul(po[:], sumC[:, sl], wsums[:], start=True, stop=False)
                nc.tensor.matmul(po[:], maxC[:, sl], wmaxs[:], start=False, stop=False)
                nc.tensor.matmul(po[:], ones[:], bos[:], start=False, stop=True)

                pm = pso.tile([H, O], f32, tag="pm")
                nc.tensor.matmul(pm[:], sumC[:, sl], wmeans[:], start=True, stop=True)

                om = h1pool.tile([H, O], f32, tag="om")
                nc.vector.tensor_scalar_mul(om[:], pm[:], recs[:, ch : ch + 1])
                ot = h2pool.tile([H, O], f32, tag="ot")
                nc.vector.tensor_tensor(ot[:], po[:], om[:], op=mybir.AluOpType.add)
                nc.sync.dma_start(out[sl, :], ot[:])

    nc.compile()
    return nc


def kernel(**inputs):
    x = np.ascontiguousarray(np.asarray(inputs["x"], dtype=np.float32))
    batch = np.asarray(inputs["batch"]).astype(np.int64)

    # ---- fold BN into the linears ----
    W1p, b1p = _fold_bn(
        np.asarray(inputs["W1"]), np.asarray(inputs["b1"]),
        np.asarray(inputs["g1"]), np.asarray(inputs["be1"]),
        np.asarray(inputs["m1"]), np.asarray(inputs["v1"]),
    )
    W2p, b2p = _fold_bn(
        np.asarray(inputs["W2"]), np.asarray(inputs["b2"]),
        np.asarray(inputs["g2"]), np.asarray(inputs["be2"]),
        np.asarray(inputs["m2"]), np.asarray(inputs["v2"]),
    )
    W3p, b3p = _fold_bn(
        np.asarray(inputs["W3"]), np.asarray(inputs["b3"]),
        np.asarray(inputs["g3"]), np.asarray(inputs["be3"]),
        np.asarray(inputs["m3"]), np.asarray(inputs["v3"]),
    )
    Wop, bop = _fold_bn(
        np.asarray(inputs["Wo"]), np.asarray(inputs["bo"]),
        np.asarray(inputs["go"]), np.asarray(inputs["beo"]),
        np.asarray(inputs["mo"]), np.asarray(inputs["vo"]),
    )

    # bf16 copies of the streaming-path weights (also used for the BIG bound)
    W1b = W1p.astype(BF16).astype(np.float32)
    W2b = W2p.astype(BF16).astype(np.float32)
    W3b = W3p.astype(BF16).astype(np.float32)

    # Pad columns are zero in x, so h2_pad is a known constant; BIG pushes the
    # padded layer-3 pre-activation strictly below zero (with margin for the
    # bf16 rounding of x/h tiles).
    h1_pad = np.maximum(b1p, 0.0)
    h2_pad = np.maximum(W2b @ h1_pad + b2p, 0.0)
    v3 = W3b @ h2_pad + b3p
    BIG = float(max(0.0, v3.max()) + 1000.0)

    # ---- whole-segment sharding by sorted-width round-robin rank ----
    counts = np.bincount(batch, minlength=NSEG).astype(np.int64)
    assert np.all(batch[:-1] <= batch[1:]), "batch must be sorted"
    order = np.argsort(-counts, kind="stable")  # segment ids, width desc
    slot_w = np.maximum(counts[order[::NCORES][:S]], 1)  # width of rank 8k
    tiles0, cols = _plan_tiles(slot_w)

    # per-tile tail-window width: the trailing columns of each slot that can
    # be padding on ANY core (only these need the -BIG mask matmul)
    wmat = counts[order[: S * NCORES]].reshape(S, NCORES)  # slot x core widths
    tiles = []
    for k0, d, wt, col0 in tiles0:
        minw = int(wmat[k0 : k0 + d].min())
        tailw = min(wt, (wt - minw + 1) & ~1)
        tiles.append((k0, d, wt, col0, tailw))

    key = (cols, float(BIG), tuple(slot_w.tolist()),
           tuple(t[4] for t in tiles), ACT_SLOTS)
    if key not in _compiled_cache:
        _compiled_cache[key] = _build_program(tiles, cols, BIG)
    nc = _compiled_cache[key]

    # column start and padded width of each slot
    slot_col = np.zeros(S, dtype=np.int64)
    slot_wt = np.zeros(S, dtype=np.int64)
    for k0, d, wt, col0, tailw in tiles:
        for j in range(d):
            slot_col[k0 + j] = col0 + j * wt
            slot_wt[k0 + j] = wt
    # epilogue shift corrections (DVE-shifted slots only): every window
    # column contributes -b3 of shift, pads included, so the correction is
    # +wt*b3 for sums and +b3 for maxes; ACT-fused slots need none.
    shifted = (np.arange(S) >= ACT_SLOTS).astype(np.float32)
    wtb3_host = np.ascontiguousarray(
        b3p[:, None] * (slot_wt.astype(np.float32) * shifted)[None, :]
    ).astype(np.float32)
    mb3_host = np.ascontiguousarray(
        b3p[:, None] * shifted[None, :]
    ).astype(np.float32)

    starts = np.searchsorted(batch, np.arange(NSEG), side="left")
    ends = np.searchsorted(batch, np.arange(NSEG), side="right")

    in_maps = []
    for c in range(NCORES):
        segs = order[np.arange(S) * NCORES + c]  # this core's segment ids
        src = np.full(cols, -1, dtype=np.int64)
        for k in range(S):
            s = segs[k]
            cnt = int(counts[s])
            if cnt:
                src[slot_col[k] : slot_col[k] + cnt] = np.arange(
                    starts[s], ends[s]
                )
        real = src >= 0
        xTc = np.zeros((C, cols), dtype=BF16)
        xTc[:, real] = x[src[real]].T.astype(BF16)
        auxc = np.zeros((1, cols), dtype=BF16)
        auxc[0, ~real] = 1.0
        recipc = (1.0 / np.maximum(counts[segs], 1.0)).astype(np.float32)
        in_maps.append(
            dict(
                xT=xTc,
                aux=auxc,
                w1=np.ascontiguousarray(W1p.T.astype(BF16)),
                w2=np.ascontiguousarray(W2p.T.astype(BF16)),
                w3=np.ascontiguousarray(W3p.T.astype(BF16)),
                b1=np.ascontiguousarray(b1p[:, None]),
                b2=np.ascontiguousarray(b2p[:, None]),
                b3=np.ascontiguousarray(b3p[:, None]),
                nbig=np.full((1, H), -BIG, BF16),
                wsum=np.ascontiguousarray(Wop[:, 0:H].T),
                wmax=np.ascontiguousarray(Wop[:, H : 2 * H].T),
                wmean=np.ascontiguousarray(Wop[:, 2 * H : 3 * H].T),
                bo=np.ascontiguousarray(bop[None, :]),
                recip=np.ascontiguousarray(recipc.reshape(S // H, H).T),
                nb3=np.ascontiguousarray(-b3p[:, None]),
                wtb3=wtb3_host,
                mb3=mb3_host,
            )
        )

    ncores_run = int(os.environ.get("KERNEL_NCORES", str(NCORES)))
    res = bass_utils.run_bass_kernel_spmd(
        nc,
        in_maps[:ncores_run],
        core_ids=list(range(ncores_run)),
        trace=bool(int(os.environ.get("KERNEL_TRACE", "0"))),
        tmpdir=os.environ.get("KERNEL_TRACE_DIR") or None,
    )
    kernel.last_results = res

    out_full = np.zeros((NSEG, O), dtype=np.float32)
    ranks = np.arange(S)
    for c in range(ncores_run):
        out_full[order[ranks * NCORES + c]] = res.results[c]["out"]
    return out_full


# revision 30
# speedup vs baseline: 1.3332x; 1.1183x over previous
"""DeepSets segment-reduce kernel for 8x Trainium2 NeuronCores.

Strategy (all shapes hardcoded for N=500000, C=H=128, O=64, NSEG=2048):
  - Transposed activation layout: features on SBUF partitions, nodes on the
    free axis, so segment reductions are free-axis DVE ops.
  - Whole-segment sharding: every segment is assigned entirely to one core,
    round-robin by global sorted-width rank.  All 8 cores then share an
    identical compile-time slot/tile geometry (SPMD-safe); per-core padding
    is <1%.  No collective is needed - the host gather is the unshard.
  - Encoder BN is folded into the linear weights (W' = W * g*rsqrt(v+eps),
    b' = (b-m)*g*rsqrt(v+eps) + beta), so each layer is relu(W'x + b').
  - bf16 everywhere on the streaming path (x, weights, h tiles) for
    1 cycle/row matmuls, FWL weight loads and half the HBM traffic; PSUM
    stays fp32 (TRN2), biases stay fp32 via the ACT bias / TS scalar port.
  - A large negative pad mask is injected into layer-3 PSUM by a rank-1
    matmul (-BIG x is_pad) over the per-slot tail windows.  Pad columns
    fall below zero, so they contribute exactly 0 to post-relu segment
    sums and never win the post-relu segment max.
  - Engine balance: ACT does relu1+relu2 (PSUM->SBUF, bias via ACT port).
    DVE does a fused per-slot tensor_scalar for layer 3 using the shift
    identity  g3 = max(z3, -b3) = relu(z3+b3) - b3:  out=g3 (bf16 SBUF)
    and accum_out = sum(g3 window)  (TS semantics: out = in0 op0 s1,
    accum_out = reduce(out, op1)).  One reduce_max per tile over g3 gives
    the shifted segment max.  The shifts are exact compile-time
    constants, fixed in the epilogue:  sum += wt*b3  (wt = padded slot
    width; every pad/empty column contributes exactly -b3) and
    max += b3 (all-pad slots come out at -b3, i.e. 0 after the shift,
    matching the reference's empty-segment zero).
  - Final projection out = [sum|max|mean] @ Wo'.T + bo' runs per core on
    its own 256 segments; mean is handled by projecting sums through the
    mean block of Wo' and row-scaling by 1/count.
"""

import os
import sys

import numpy as np

if "/opt/trn_rl_repo" not in sys.path:
    sys.path.insert(0, "/opt/trn_rl_repo")

import ml_dtypes

import concourse.bacc as bacc
import concourse.mybir as mybir
import concourse.tile as tile
from concourse import bass_utils

EPS = 1e-5
NSEG = 2048
NCORES = 8
C = 128
H = 128
O = 64
S = NSEG // NCORES  # segment slots per core (256)
MAX_TILE = 512  # PSUM bank / moving-operand limit
SB_COLS = 2048  # superblock width: one xT DMA covers several tiles
# GPSIMD cannot run TENSOR_TENSOR on TRN2 (no such opcode on Pool) — keep 0.
MAX_GPSIMD = int(os.environ.get("KERNEL_MAX_GPSIMD", "0"))
# Number of slots whose fused relu3+sum runs on ACT (activation accum_out,
# true relu) instead of DVE (shift-trick tensor_scalar) — load balancing.
ACT_SLOTS = int(os.environ.get("KERNEL_ACT_SLOTS", "36"))

BF16 = ml_dtypes.bfloat16

_compiled_cache = {}


def _fold_bn(W, b, g, be, m, v):
    a = g / np.sqrt(v + EPS)
    Wp = W * a[:, None]
    bp = (b - m) * a + be
    return Wp.astype(np.float32), bp.astype(np.float32)


def _plan_tiles(slot_w):
    """Greedy-pack slots (widths descending) into tiles of <=MAX_TILE cols.

    Returns list of (slot_start, n_slots, padded_width, col_start) and the
    total padded column count.
    """
    tiles = []
    col = 0
    k = 0
    n = len(slot_w)
    while k < n:
        wt = (int(slot_w[k]) + 1) & ~1  # even widths keep windows 4B-aligned
        assert 0 < wt <= MAX_TILE, f"slot width {wt} unsupported"
        d = min(MAX_TILE // wt, n - k)
        tiles.append((k, d, wt, col))
        col += d * wt
        k += d
    return tiles, col


def _build_program(tiles, cols, BIG_DEVICE):
    """Emit the Bass/Tile program shared by all 8 cores."""
    nc = bacc.Bacc(
        "TRN2",
        target_bir_lowering=False,
        debug=False,
        num_devices=NCORES,
    )
    f32 = mybir.dt.float32
    bf16 = mybir.dt.bfloat16

    xT = nc.dram_tensor("xT", [C, cols], bf16, kind="ExternalInput").ap()
    aux = nc.dram_tensor("aux", [1, cols], bf16, kind="ExternalInput").ap()
    w1 = nc.dram_tensor("w1", [C, H], bf16, kind="ExternalInput").ap()
    w2 = nc.dram_tensor("w2", [H, H], bf16, kind="ExternalInput").ap()
    w3 = nc.dram_tensor("w3", [H, H], bf16, kind="ExternalInput").ap()
    b1 = nc.dram_tensor("b1", [H, 1], f32, kind="ExternalInput").ap()
    b3 = nc.dram_tensor("b3", [H, 1], f32, kind="ExternalInput").ap()
    nbig = nc.dram_tensor("nbig", [1, H], bf16, kind="ExternalInput").ap()
    b2 = nc.dram_tensor("b2", [H, 1], f32, kind="ExternalInput").ap()
    wsum = nc.dram_tensor("wsum", [H, O], f32, kind="ExternalInput").ap()
    wmax = nc.dram_tensor("wmax", [H, O], f32, kind="ExternalInput").ap()
    wmean = nc.dram_tensor("wmean", [H, O], f32, kind="ExternalInput").ap()
    bo = nc.dram_tensor("bo", [1, O], f32, kind="ExternalInput").ap()
    # column ch holds the reciprocals for segment chunk ch (128 slots each)
    recip = nc.dram_tensor("recip", [H, S // H], f32, kind="ExternalInput").ap()
    nb3 = nc.dram_tensor("nb3", [H, 1], f32, kind="ExternalInput").ap()
    wtb3 = nc.dram_tensor("wtb3", [H, S], f32, kind="ExternalInput").ap()
    mb3 = nc.dram_tensor("mb3", [H, S], f32, kind="ExternalInput").ap()
    out = nc.dram_tensor("out", [S, O], f32, kind="ExternalOutput").ap()

    # group consecutive tiles into superblocks sharing one xT/aux DMA
    sblocks = []
    cur, cur_col0, cur_cols = [], 0, 0
    for t in tiles:
        _k0, _d, _wt, _col0, _tailw = t
        _tcols = _d * _wt
        if cur and (_col0 + _tcols - cur_col0) > SB_COLS:
            sblocks.append((cur_col0, cur_cols, cur))
            cur = []
        if not cur:
            cur_col0 = _col0
        cur.append(t)
        cur_cols = _col0 + _tcols - cur_col0
    if cur:
        sblocks.append((cur_col0, cur_cols, cur))

    with tile.TileContext(nc) as tc:
        with (
            tc.tile_pool(name="const", bufs=1) as cpool,
            tc.tile_pool(name="xin", bufs=3) as xpool,
            tc.tile_pool(name="auxin", bufs=3) as apool,
            tc.tile_pool(name="h1", bufs=4) as h1pool,
            tc.tile_pool(name="h2", bufs=4) as h2pool,
            tc.tile_pool(name="h3", bufs=4) as h3pool,
            tc.tile_pool(name="gh", bufs=4) as ghpool,
            tc.tile_pool(name="acc", bufs=1) as accpool,
            tc.tile_pool(name="ps1", bufs=2, space="PSUM") as ps1,
            tc.tile_pool(name="ps2", bufs=2, space="PSUM") as ps2,
            tc.tile_pool(name="ps3", bufs=2, space="PSUM") as ps3,
            tc.tile_pool(name="pso", bufs=1, space="PSUM") as pso,
        ):
            w1s = cpool.tile([C, H], bf16, tag="w1")
            w2s = cpool.tile([H, H], bf16, tag="w2")
            w3s = cpool.tile([H, H], bf16, tag="w3")
            b1s = cpool.tile([H, 1], f32, tag="b1")
            b3s = cpool.tile([H, 1], f32, tag="b3")
            negbig = cpool.tile([1, H], bf16, tag="negbig")
            b2s = cpool.tile([H, 1], f32, tag="b2")
            wsums = cpool.tile([H, O], f32, tag="wsum")
            wmaxs = cpool.tile([H, O], f32, tag="wmax")
            wmeans = cpool.tile([H, O], f32, tag="wmean")
            bos = cpool.tile([1, O], f32, tag="bo")
            recs = cpool.tile([H, S // H], f32, tag="recip")
            ones = cpool.tile([1, H], f32, tag="ones")
            nb3s = cpool.tile([H, 1], f32, tag="nb3")
            wtb3s = cpool.tile([H, S], f32, tag="wtb3")
            mb3s = cpool.tile([H, S], f32, tag="mb3")

            # constants go through the PE/ACT/DVE DMA queues so the Sync
            # queue can start streaming xT superblocks immediately
            nc.tensor.dma_start(w1s[:], w1)
            nc.tensor.dma_start(w2s[:], w2)
            nc.tensor.dma_start(w3s[:], w3)
            nc.tensor.dma_start(b1s[:], b1)
            nc.scalar.dma_start(b3s[:], b3)
            nc.scalar.dma_start(b2s[:], b2)
            nc.scalar.dma_start(wsums[:], wsum)
            nc.scalar.dma_start(wmaxs[:], wmax)
            nc.vector.dma_start(wmeans[:], wmean)
            nc.vector.dma_start(bos[:], bo)
            nc.vector.dma_start(recs[:], recip)
            nc.vector.memset(ones[:], 1.0)
            nc.tensor.dma_start(negbig[:], nbig)
            nc.vector.dma_start(nb3s[:], nb3)
            nc.scalar.dma_start(wtb3s[:], wtb3)
            nc.scalar.dma_start(mb3s[:], mb3)

            # Persistent per-slot partials (both post-relu, bias included).
            sumP = accpool.tile([H, S], f32, tag="sumP")
            maxP = accpool.tile([H, S], f32, tag="maxP")

            relu = mybir.ActivationFunctionType.Relu
            add = mybir.AluOpType.add
            amax = mybir.AluOpType.max

            for sb_col0, sb_cols, sbtiles in sblocks:
                xt = xpool.tile([C, SB_COLS], bf16, tag="xt")
                nc.sync.dma_start(
                    xt[:, :sb_cols], xT[:, sb_col0 : sb_col0 + sb_cols]
                )
                need_aux = any(t[4] > 0 for t in sbtiles)
                if need_aux:
                    at = apool.tile([1, SB_COLS], bf16, tag="at")
                    nc.sync.dma_start(
                        at[:, :sb_cols], aux[:, sb_col0 : sb_col0 + sb_cols]
                    )

                # one g3 buffer per superblock so same-width neighbor tiles
                # can share a single reduce_max instruction
                g3 = h3pool.tile([H, SB_COLS], bf16, tag="g3")

                for k0, d, wt, col0, tailw in sbtiles:
                    tcols = d * wt
                    base = col0 - sb_col0
                    xts = xt[:, base : base + tcols]

                    p1 = ps1.tile([H, MAX_TILE], f32, tag="p1")
                    nc.tensor.matmul(p1[:, :tcols], w1s[:], xts)
                    h1 = h1pool.tile([H, MAX_TILE], bf16, tag="h1")
                    nc.scalar.activation(
                        h1[:, :tcols], p1[:, :tcols], relu, bias=b1s[:]
                    )

                    p2 = ps2.tile([H, MAX_TILE], f32, tag="p2")
                    nc.tensor.matmul(p2[:, :tcols], w2s[:], h1[:, :tcols])
                    h2 = h2pool.tile([H, MAX_TILE], bf16, tag="h2")
                    nc.scalar.activation(
                        h2[:, :tcols], p2[:, :tcols], relu, bias=b2s[:]
                    )

                    p3 = ps3.tile([H, MAX_TILE], f32, tag="p3")
                    p3v = p3[:, :tcols].rearrange("p (d w) -> p d w", d=d)
                    if tailw > 0:
                        nc.tensor.matmul(
                            p3[:, :tcols], w3s[:], h2[:, :tcols],
                            start=True, stop=False,
                        )
                        # -BIG into the per-slot tail windows (the only
                        # columns that can be pads on any core).
                        atv = at[:, base : base + tcols].rearrange(
                            "p (d w) -> p d w", d=d
                        )
                        nc.tensor.matmul(
                            p3v[:, :, wt - tailw : wt],
                            negbig[:],
                            atv[:, :, wt - tailw : wt],
                            start=False,
                            stop=True,
                        )
                    else:
                        nc.tensor.matmul(
                            p3[:, :tcols], w3s[:], h2[:, :tcols],
                            start=True, stop=True,
                        )

                    # Fused per-slot layer-3 relu + segment sum.  Slot
                    # k < ACT_SLOTS runs on ACT (true relu+bias, accum =
                    # true sum, no correction); the rest run on DVE with
                    # the shift identity g3 = max(z3,-b3) = relu(z3+b3)-b3
                    # (corrected by +wt*b3 / +b3 in the epilogue).
                    for j in range(d):
                        c0 = base + j * wt
                        k = k0 + j
                        if k < ACT_SLOTS:
                            nc.scalar.activation(
                                g3[:, c0 : c0 + wt],
                                p3[:, j * wt : j * wt + wt],
                                relu,
                                bias=b3s[:],
                                accum_out=sumP[:, k : k + 1],
                            )
                        else:
                            nc.vector.tensor_scalar(
                                g3[:, c0 : c0 + wt],
                                p3[:, j * wt : j * wt + wt],
                                nb3s[:],
                                0.0,
                                op0=amax,
                                op1=add,
                                accum_out=sumP[:, k : k + 1],
                            )

                # one reduce_max per run of equal-width tiles (they are
                # column-adjacent in g3, so a single 3D access pattern
                # covers all their slot windows)
                ri = 0
                while ri < len(sbtiles):
                    k0r, dr, wtr, col0r, _ = sbtiles[ri]
                    D = dr
                    rj = ri + 1
                    while rj < len(sbtiles) and sbtiles[rj][2] == wtr:
                        D += sbtiles[rj][1]
                        rj += 1
                    baser = col0r - sb_col0
                    g3v = g3[:, baser : baser + D * wtr].rearrange(
                        "p (d w) -> p d w", d=D
                    )
                    nc.vector.reduce_max(
                        maxP[:, k0r : k0r + D], g3v, axis=mybir.AxisListType.X
                    )
                    ri = rj

            # ---- epilogue: undo the b3 shift, then project ----
            # sumC = sumP + wt*b3 (true post-relu sums); maxC = maxP + b3
            # (true post-relu maxes; all-pad slots -> exactly 0).
            sumC = accpool.tile([H, S], f32, tag="sumC")
            maxC = accpool.tile([H, S], f32, tag="maxC")
            nc.vector.tensor_tensor(sumC[:], sumP[:], wtb3s[:], op=add)
            nc.vector.tensor_tensor(maxC[:], maxP[:], mb3s[:], op=add)

            # out[k, :] = sum_k @ Wsum + max_k @ Wmax
            #             + (sum_k * recip_k) @ Wmean + bo
            for ch in range(S // H):  # 2 chunks of 128 segments
                sl = slice(ch * H, (ch + 1) * H)
                po = pso.tile([H, O], f32, tag="po")
                nc.tensor.matmul(po[:], sumC[:, sl], wsums[:], start=True, stop=False)
                nc.tensor.matmul(po[:], maxC[:, sl], wmaxs[:], start=False, stop=False)
                nc.tensor.matmul(po[:], ones[:], bos[:], start=False, stop=True)

                pm = pso.tile([H, O], f32, tag="pm")
                nc.tensor.matmul(pm[:], sumC[:, sl], wmeans[:], start=True, stop=True)

                om = h1pool.tile([H, O], f32, tag="om")
                nc.vector.tensor_scalar_mul(om[:], pm[:], recs[:, ch : ch + 1])
                ot = h2pool.tile([H, O], f32, tag="ot")
                nc.vector.tensor_tensor(ot[:], po[:], om[:], op=mybir.AluOpType.add)
                nc.sync.dma_start(out[sl, :], ot[:])

    nc.compile()
    return nc


def kernel(**inputs):
    x = np.ascontiguousarray(np.asarray(inputs["x"], dtype=np.float32))
    batch = np.asarray(inputs["batch"]).astype(np.int64)

    # ---- fold BN into the linears ----
    W1p, b1p = _fold_bn(
        np.asarray(inputs["W1"]), np.asarray(inputs["b1"]),
        np.asarray(inputs["g1"]), np.asarray(inputs["be1"]),
        np.asarray(inputs["m1"]), np.asarray(inputs["v1"]),
    )
    W2p, b2p = _fold_bn(
        np.asarray(inputs["W2"]), np.asarray(inputs["b2"]),
        np.asarray(inputs["g2"]), np.asarray(inputs["be2"]),
        np.asarray(inputs["m2"]), np.asarray(inputs["v2"]),
    )
    W3p, b3p = _fold_bn(
        np.asarray(inputs["W3"]), np.asarray(inputs["b3"]),
        np.asarray(inputs["g3"]), np.asarray(inputs["be3"]),
        np.asarray(inputs["m3"]), np.asarray(inputs["v3"]),
    )
    Wop, bop = _fold_bn(
        np.asarray(inputs["Wo"]), np.asarray(inputs["bo"]),
        np.asarray(inputs["go"]), np.asarray(inputs["beo"]),
        np.asarray(inputs["mo"]), np.asarray(inputs["vo"]),
    )

    # bf16 copies of the streaming-path weights (also used for the BIG bound)
    W1b = W1p.astype(BF16).astype(np.float32)
    W2b = W2p.astype(BF16).astype(np.float32)
    W3b = W3p.astype(BF16).astype(np.float32)

    # Pad columns are zero in x, so h2_pad is a known constant; BIG pushes the
    # padded layer-3 pre-activation strictly below zero (with margin for the
    # bf16 rounding of x/h tiles).
    h1_pad = np.maximum(b1p, 0.0)
    h2_pad = np.maximum(W2b @ h1_pad + b2p, 0.0)
    v3 = W3b @ h2_pad + b3p
    BIG = float(max(0.0, v3.max()) + 1000.0)

    # ---- whole-segment sharding by sorted-width round-robin rank ----
    counts = np.bincount(batch, minlength=NSEG).astype(np.int64)
    assert np.all(batch[:-1] <= batch[1:]), "batch must be sorted"
    order = np.argsort(-counts, kind="stable")  # segment ids, width desc
    slot_w = np.maximum(counts[order[::NCORES][:S]], 1)  # width of rank 8k
    tiles0, cols = _plan_tiles(slot_w)

    # per-tile tail-window width: the trailing columns of each slot that can
    # be padding on ANY core (only these need the -BIG mask matmul)
    wmat = counts[order[: S * NCORES]].reshape(S, NCORES)  # slot x core widths
    tiles = []
    for k0, d, wt, col0 in tiles0:
        minw = int(wmat[k0 : k0 + d].min())
        tailw = min(wt, (wt - minw + 1) & ~1)
        tiles.append((k0, d, wt, col0, tailw))

    key = (cols, float(BIG), tuple(slot_w.tolist()),
           tuple(t[4] for t in tiles), ACT_SLOTS)
    if key not in _compiled_cache:
        _compiled_cache[key] = _build_program(tiles, cols, BIG)
    nc = _compiled_cache[key]

    # column start and padded width of each slot
    slot_col = np.zeros(S, dtype=np.int64)
    slot_wt = np.zeros(S, dtype=np.int64)
    for k0, d, wt, col0, tailw in tiles:
        for j in range(d):
            slot_col[k0 + j] = col0 + j * wt
            slot_wt[k0 + j] = wt
    # epilogue shift corrections (DVE-shifted slots only): every window
    # column contributes -b3 of shift, pads included, so the correction is
    # +wt*b3 for sums and +b3 for maxes; ACT-fused slots need none.
    shifted = (np.arange(S) >= ACT_SLOTS).astype(np.float32)
    wtb3_host = np.ascontiguousarray(
        b3p[:, None] * (slot_wt.astype(np.float32) * shifted)[None, :]
    ).astype(np.float32)
    mb3_host = np.ascontiguousarray(
        b3p[:, None] * shifted[None, :]
    ).astype(np.float32)

    starts = np.searchsorted(batch, np.arange(NSEG), side="left")
    ends = np.searchsorted(batch, np.arange(NSEG), side="right")

    in_maps = []
    for c in range(NCORES):
        segs = order[np.arange(S) * NCORES + c]  # this core's segment ids
        src = np.full(cols, -1, dtype=np.int64)
        for k in range(S):
            s = segs[k]
            cnt = int(counts[s])
            if cnt:
                src[slot_col[k] : slot_col[k] + cnt] = np.arange(
                    starts[s], ends[s]
                )
        real = src >= 0
        xTc = np.zeros((C, cols), dtype=BF16)
        xTc[:, real] = x[src[real]].T.astype(BF16)
        auxc = np.zeros((1, cols), dtype=BF16)
        auxc[0, ~real] = 1.0
        recipc = (1.0 / np.maximum(counts[segs], 1.0)).astype(np.float32)
        in_maps.append(
            dict(
                xT=xTc,
                aux=auxc,
                w1=np.ascontiguousarray(W1p.T.astype(BF16)),
                w2=np.ascontiguousarray(W2p.T.astype(BF16)),
                w3=np.ascontiguousarray(W3p.T.astype(BF16)),
                b1=np.ascontiguousarray(b1p[:, None]),
                b2=np.ascontiguousarray(b2p[:, None]),
                b3=np.ascontiguousarray(b3p[:, None]),
                nbig=np.full((1, H), -BIG, BF16),
                wsum=np.ascontiguousarray(Wop[:, 0:H].T),
                wmax=np.ascontiguousarray(Wop[:, H : 2 * H].T),
                wmean=np.ascontiguousarray(Wop[:, 2 * H : 3 * H].T),
                bo=np.ascontiguousarray(bop[None, :]),
                recip=np.ascontiguousarray(recipc.reshape(S // H, H).T),
                nb3=np.ascontiguousarray(-b3p[:, None]),
                wtb3=wtb3_host,
                mb3=mb3_host,
            )
        )

    ncores_run = int(os.environ.get("KERNEL_NCORES", str(NCORES)))
    res = bass_utils.run_bass_kernel_spmd(
        nc,
        in_maps[:ncores_run],
        core_ids=list(range(ncores_run)),
        trace=bool(int(os.environ.get("KERNEL_TRACE", "0"))),
        tmpdir=os.environ.get("KERNEL_TRACE_DIR") or None,
    )
    kernel.last_results = res

    out_full = np.zeros((NSEG, O), dtype=np.float32)
    ranks = np.arange(S)
    for c in range(ncores_run):
        out_full[order[ranks * NCORES + c]] = res.results[c]["out"]
    return out_full


# revision 40
# speedup vs baseline: 1.3959x; 1.0471x over previous
"""DeepSets segment-reduce kernel for 8x Trainium2 NeuronCores.

Strategy (all shapes hardcoded for N=500000, C=H=128, O=64, NSEG=2048):
  - Transposed activation layout: features on SBUF partitions, nodes on the
    free axis, so segment reductions are free-axis DVE ops.
  - Whole-segment sharding: every segment is assigned entirely to one core,
    round-robin by global sorted-width rank.  All 8 cores then share an
    identical compile-time slot/tile geometry (SPMD-safe); per-core padding
    is <1%.  No collective is needed - the host gather is the unshard.
  - Encoder BN is folded into the linear weights (W' = W * g*rsqrt(v+eps),
    b' = (b-m)*g*rsqrt(v+eps) + beta), so each layer is relu(W'x + b').
  - bf16 everywhere on the streaming path (x, weights, h tiles) for
    1 cycle/row matmuls, FWL weight loads and half the HBM traffic; PSUM
    stays fp32 (TRN2), biases stay fp32 via the ACT bias / TS scalar port.
  - A large negative pad mask is injected into layer-3 PSUM by a rank-1
    matmul (-BIG x is_pad) over the per-slot tail windows.  Pad columns
    fall below zero, so they contribute exactly 0 to post-relu segment
    sums and never win the post-relu segment max.
  - Engine balance: ACT does relu1+relu2 (PSUM->SBUF, bias via ACT port).
    DVE does a fused per-slot tensor_scalar for layer 3 using the shift
    identity  g3 = max(z3, -b3) = relu(z3+b3) - b3:  out=g3 (bf16 SBUF)
    and accum_out = sum(g3 window)  (TS semantics: out = in0 op0 s1,
    accum_out = reduce(out, op1)).  One reduce_max per tile over g3 gives
    the shifted segment max.  The shifts are exact compile-time
    constants, fixed in the epilogue:  sum += wt*b3  (wt = padded slot
    width; every pad/empty column contributes exactly -b3) and
    max += b3 (all-pad slots come out at -b3, i.e. 0 after the shift,
    matching the reference's empty-segment zero).
  - Final projection out = [sum|max|mean] @ Wo'.T + bo' runs per core on
    its own 256 segments; mean is handled by projecting sums through the
    mean block of Wo' and row-scaling by 1/count.
"""

import os
import sys

import numpy as np

if "/opt/trn_rl_repo" not in sys.path:
    sys.path.insert(0, "/opt/trn_rl_repo")

import ml_dtypes

import concourse.bacc as bacc
import concourse.bass as bass_mod
import concourse.mybir as mybir
import concourse.tile as tile
from concourse import bass_utils

EPS = 1e-5
NSEG = 2048
NCORES = 8
C = 128
H = 128
O = 64
S = NSEG // NCORES  # segment slots per core (256)
MAX_TILE = 512  # PSUM bank / moving-operand limit
SB_COLS = 2048  # superblock width: one xT DMA covers several tiles
# GPSIMD cannot run TENSOR_TENSOR on TRN2 (no such opcode on Pool) — keep 0.
MAX_GPSIMD = int(os.environ.get("KERNEL_MAX_GPSIMD", "0"))
# Number of slots whose fused relu3+sum runs on ACT (activation accum_out,
# true relu) instead of DVE (shift-trick tensor_scalar) — load balancing.
ACT_SLOTS = int(os.environ.get("KERNEL_ACT_SLOTS", "0"))
# Segment max engine: TRN2 walrus rejects InstPool on GPSIMD, keep "dve".
MAXENG = os.environ.get("KERNEL_MAXENG", "dve")  # "pool" | "dve"
# Every Nth tile's relu2 runs on DVE (tensor_scalar) to balance ACT; 0 = off.
R2DVE = int(os.environ.get("KERNEL_R2DVE", "0"))

BF16 = ml_dtypes.bfloat16

_compiled_cache = {}


def _fold_bn(W, b, g, be, m, v):
    a = g / np.sqrt(v + EPS)
    Wp = W * a[:, None]
    bp = (b - m) * a + be
    return Wp.astype(np.float32), bp.astype(np.float32)


def _plan_tiles(slot_w):
    """Greedy-pack slots (widths descending) into tiles of <=MAX_TILE cols.

    Returns list of (slot_start, n_slots, padded_width, col_start) and the
    total padded column count.
    """
    tiles = []
    col = 0
    k = 0
    n = len(slot_w)
    while k < n:
        wt = (int(slot_w[k]) + 1) & ~1  # even widths keep windows 4B-aligned
        assert 0 < wt <= MAX_TILE, f"slot width {wt} unsupported"
        d = min(MAX_TILE // wt, n - k)
        tiles.append((k, d, wt, col))
        col += d * wt
        k += d
    return tiles, col


def _build_program(tiles, cols, BIG_DEVICE):
    """Emit the Bass/Tile program shared by all 8 cores."""
    nc = bacc.Bacc(
        "TRN2",
        target_bir_lowering=False,
        debug=False,
        num_devices=NCORES,
    )
    f32 = mybir.dt.float32
    bf16 = mybir.dt.bfloat16

    xT = nc.dram_tensor("xT", [C, cols], bf16, kind="ExternalInput").ap()
    aux = nc.dram_tensor("aux", [1, cols], bf16, kind="ExternalInput").ap()
    w1 = nc.dram_tensor("w1", [C, H], bf16, kind="ExternalInput").ap()
    w2 = nc.dram_tensor("w2", [H, H], bf16, kind="ExternalInput").ap()
    w3 = nc.dram_tensor("w3", [H, H], bf16, kind="ExternalInput").ap()
    b1 = nc.dram_tensor("b1", [H, 1], f32, kind="ExternalInput").ap()
    b3 = nc.dram_tensor("b3", [H, 1], f32, kind="ExternalInput").ap()
    nbig = nc.dram_tensor("nbig", [1, H], bf16, kind="ExternalInput").ap()
    b2 = nc.dram_tensor("b2", [H, 1], f32, kind="ExternalInput").ap()
    wsum = nc.dram_tensor("wsum", [H, O], f32, kind="ExternalInput").ap()
    wmax = nc.dram_tensor("wmax", [H, O], f32, kind="ExternalInput").ap()
    wmean = nc.dram_tensor("wmean", [H, O], f32, kind="ExternalInput").ap()
    bo = nc.dram_tensor("bo", [1, O], f32, kind="ExternalInput").ap()
    # column ch holds the reciprocals for segment chunk ch (128 slots each)
    recip = nc.dram_tensor("recip", [H, S // H], f32, kind="ExternalInput").ap()
    nb3 = nc.dram_tensor("nb3", [H, 1], f32, kind="ExternalInput").ap()
    wtb3 = nc.dram_tensor("wtb3", [H, S], f32, kind="ExternalInput").ap()
    mb3 = nc.dram_tensor("mb3", [H, S], f32, kind="ExternalInput").ap()
    out = nc.dram_tensor("out", [S, O], f32, kind="ExternalOutput").ap()

    # group consecutive tiles into superblocks sharing one xT/aux DMA
    sblocks = []
    cur, cur_col0, cur_cols = [], 0, 0
    for t in tiles:
        _k0, _d, _wt, _col0, _tailw = t
        _tcols = _d * _wt
        if cur and (_col0 + _tcols - cur_col0) > SB_COLS:
            sblocks.append((cur_col0, cur_cols, cur))
            cur = []
        if not cur:
            cur_col0 = _col0
        cur.append(t)
        cur_cols = _col0 + _tcols - cur_col0
    if cur:
        sblocks.append((cur_col0, cur_cols, cur))

    with tile.TileContext(nc) as tc:
        with (
            tc.tile_pool(name="const", bufs=1) as cpool,
            tc.tile_pool(name="xin", bufs=3) as xpool,
            tc.tile_pool(name="auxin", bufs=3) as apool,
            tc.tile_pool(name="h1", bufs=4) as h1pool,
            tc.tile_pool(name="h2", bufs=4) as h2pool,
            tc.tile_pool(name="h3", bufs=3) as h3pool,
            tc.tile_pool(name="acc", bufs=1) as accpool,
            tc.tile_pool(name="ps1", bufs=2, space="PSUM") as ps1,
            tc.tile_pool(name="ps2", bufs=2, space="PSUM") as ps2,
            tc.tile_pool(name="ps3", bufs=2, space="PSUM") as ps3,
            tc.tile_pool(name="pso", bufs=1, space="PSUM") as pso,
        ):
            w1s = cpool.tile([C, H], bf16, tag="w1")
            w2s = cpool.tile([H, H], bf16, tag="w2")
            w3s = cpool.tile([H, H], bf16, tag="w3")
            b1s = cpool.tile([H, 1], f32, tag="b1")
            b3s = cpool.tile([H, 1], f32, tag="b3")
            negbig = cpool.tile([1, H], bf16, tag="negbig")
            b2s = cpool.tile([H, 1], f32, tag="b2")
            wsums = cpool.tile([H, O], f32, tag="wsum")
            wmaxs = cpool.tile([H, O], f32, tag="wmax")
            wmeans = cpool.tile([H, O], f32, tag="wmean")
            bos = cpool.tile([1, O], f32, tag="bo")
            recs = cpool.tile([H, S // H], f32, tag="recip")
            ones = cpool.tile([1, H], f32, tag="ones")
            nb3s = cpool.tile([H, 1], f32, tag="nb3")
            wtb3s = cpool.tile([H, S], f32, tag="wtb3")
            mb3s = cpool.tile([H, S], f32, tag="mb3")

            # constants go through the ACT/GPSIMD DMA queues so the Sync
            # queue can start streaming xT superblocks immediately
            nc.scalar.dma_start(w1s[:], w1)
            nc.scalar.dma_start(w2s[:], w2)
            nc.scalar.dma_start(w3s[:], w3)
            nc.scalar.dma_start(b1s[:], b1)
            nc.scalar.dma_start(b3s[:], b3)
            nc.scalar.dma_start(b2s[:], b2)
            nc.gpsimd.dma_start(wsums[:], wsum)
            nc.gpsimd.dma_start(wmaxs[:], wmax)
            nc.gpsimd.dma_start(wmeans[:], wmean)
            nc.gpsimd.dma_start(bos[:], bo)
            nc.gpsimd.dma_start(recs[:], recip)
            nc.vector.memset(ones[:], 1.0)
            nc.scalar.dma_start(negbig[:], nbig)
            nc.scalar.dma_start(nb3s[:], nb3)
            nc.gpsimd.dma_start(wtb3s[:], wtb3)
            nc.gpsimd.dma_start(mb3s[:], mb3)

            # Persistent per-slot partials (both post-relu, bias included).
            sumP = accpool.tile([H, S], f32, tag="sumP")
            maxP = accpool.tile([H, S], f32, tag="maxP")

            relu = mybir.ActivationFunctionType.Relu
            add = mybir.AluOpType.add
            amax = mybir.AluOpType.max

            ti2 = 0  # global tile counter for the relu2 engine split
            for sb_col0, sb_cols, sbtiles in sblocks:
                xt = xpool.tile([C, SB_COLS], bf16, tag="xt")
                nc.sync.dma_start(
                    xt[:, :sb_cols], xT[:, sb_col0 : sb_col0 + sb_cols]
                )
                need_aux = any(t[4] > 0 for t in sbtiles)
                if need_aux:
                    at = apool.tile([1, SB_COLS], bf16, tag="at")
                    nc.sync.dma_start(
                        at[:, :sb_cols], aux[:, sb_col0 : sb_col0 + sb_cols]
                    )

                # one g3 buffer per superblock so same-width neighbor tiles
                # can share a single reduce_max instruction
                g3 = h3pool.tile([H, SB_COLS], bf16, tag="g3")

                for k0, d, wt, col0, tailw in sbtiles:
                    tcols = d * wt
                    base = col0 - sb_col0
                    xts = xt[:, base : base + tcols]

                    p1 = ps1.tile([H, MAX_TILE], f32, tag="p1")
                    nc.tensor.matmul(p1[:, :tcols], w1s[:], xts)
                    h1 = h1pool.tile([H, MAX_TILE], bf16, tag="h1")
                    nc.scalar.activation(
                        h1[:, :tcols], p1[:, :tcols], relu, bias=b1s[:]
                    )

                    p2 = ps2.tile([H, MAX_TILE], f32, tag="p2")
                    nc.tensor.matmul(p2[:, :tcols], w2s[:], h1[:, :tcols])
                    h2 = h2pool.tile([H, MAX_TILE], bf16, tag="h2")
                    if R2DVE and ti2 % R2DVE == 0:
                        nc.vector.tensor_scalar(
                            h2[:, :tcols], p2[:, :tcols], b2s[:], 0.0,
                            op0=add, op1=amax,
                        )
                    else:
                        nc.scalar.activation(
                            h2[:, :tcols], p2[:, :tcols], relu, bias=b2s[:]
                        )
                    ti2 += 1

                    p3 = ps3.tile([H, MAX_TILE], f32, tag="p3")
                    p3v = p3[:, :tcols].rearrange("p (d w) -> p d w", d=d)
                    if tailw > 0:
                        nc.tensor.matmul(
                            p3[:, :tcols], w3s[:], h2[:, :tcols],
                            start=True, stop=False,
                        )
                        # -BIG into the per-slot tail windows (the only
                        # columns that can be pads on any core).
                        atv = at[:, base : base + tcols].rearrange(
                            "p (d w) -> p d w", d=d
                        )
                        nc.tensor.matmul(
                            p3v[:, :, wt - tailw : wt],
                            negbig[:],
                            atv[:, :, wt - tailw : wt],
                            start=False,
                            stop=True,
                        )
                    else:
                        nc.tensor.matmul(
                            p3[:, :tcols], w3s[:], h2[:, :tcols],
                            start=True, stop=True,
                        )

                    # Fused per-slot layer-3 relu + segment sum.  Slot
                    # k < ACT_SLOTS runs on ACT (true relu+bias, accum =
                    # true sum, no correction); the rest run on DVE with
                    # the shift identity g3 = max(z3,-b3) = relu(z3+b3)-b3
                    # (corrected by +wt*b3 / +b3 in the epilogue).
                    for j in range(d):
                        c0 = base + j * wt
                        k = k0 + j
                        if k < ACT_SLOTS:
                            nc.scalar.activation(
                                g3[:, c0 : c0 + wt],
                                p3[:, j * wt : j * wt + wt],
                                relu,
                                bias=b3s[:],
                                accum_out=sumP[:, k : k + 1],
                            )
                        else:
                            nc.vector.tensor_scalar(
                                g3[:, c0 : c0 + wt],
                                p3[:, j * wt : j * wt + wt],
                                nb3s[:],
                                0.0,
                                op0=amax,
                                op1=add,
                                accum_out=sumP[:, k : k + 1],
                            )

                # one max-reduce per run of equal-width tiles (they are
                # column-adjacent in g3, so a single 3D access pattern
                # covers all their slot windows).  Runs on the otherwise
                # idle GPSIMD via the Q7 max_pool ucode when MAXENG=pool.
                ri = 0
                while ri < len(sbtiles):
                    k0r, dr, wtr, col0r, _ = sbtiles[ri]
                    D = dr
                    rj = ri + 1
                    while rj < len(sbtiles) and sbtiles[rj][2] == wtr:
                        D += sbtiles[rj][1]
                        rj += 1
                    baser = col0r - sb_col0
                    g3v = g3[:, baser : baser + D * wtr].rearrange(
                        "p (d w) -> p d w", d=D
                    )
                    if MAXENG == "pool":
                        bass_mod.BassVectorEngine.pool(
                            nc.gpsimd, maxP[:, k0r : k0r + D], g3v,
                            func=mybir.PoolFunctionType.max,
                        )
                    else:
                        nc.vector.reduce_max(
                            maxP[:, k0r : k0r + D], g3v,
                            axis=mybir.AxisListType.X,
                        )
                    ri = rj

            # ---- epilogue: undo the b3 shift, then project ----
            # sumC = sumP + wt*b3 (true post-relu sums); maxC = maxP + b3
            # (true post-relu maxes; all-pad slots -> exactly 0).  Done
            # per 128-slot chunk so chunk 0 can project while the last
            # tiles (high slot ids) are still reducing.
            sumC = accpool.tile([H, S], f32, tag="sumC")
            maxC = accpool.tile([H, S], f32, tag="maxC")

            # out[k, :] = sum_k @ Wsum + max_k @ Wmax
            #             + (sum_k * recip_k) @ Wmean + bo
            for ch in range(S // H):  # 2 chunks of 128 segments
                sl = slice(ch * H, (ch + 1) * H)
                nc.vector.tensor_tensor(
                    sumC[:, sl], sumP[:, sl], wtb3s[:, sl], op=add
                )
                nc.vector.tensor_tensor(
                    maxC[:, sl], maxP[:, sl], mb3s[:, sl], op=add
                )
                po = pso.tile([H, O], f32, tag="po")
                nc.tensor.matmul(po[:], sumC[:, sl], wsums[:], start=True, stop=False)
                nc.tensor.matmul(po[:], maxC[:, sl], wmaxs[:], start=False, stop=False)
                nc.tensor.matmul(po[:], ones[:], bos[:], start=False, stop=True)

                pm = pso.tile([H, O], f32, tag="pm")
                nc.tensor.matmul(pm[:], sumC[:, sl], wmeans[:], start=True, stop=True)

                om = h1pool.tile([H, O], f32, tag="om")
                nc.vector.tensor_scalar_mul(om[:], pm[:], recs[:, ch : ch + 1])
                ot = h2pool.tile([H, O], f32, tag="ot")
                nc.vector.tensor_tensor(ot[:], po[:], om[:], op=mybir.AluOpType.add)
                nc.sync.dma_start(out[sl, :], ot[:])

    nc.compile()
    return nc


def kernel(**inputs):
    x = np.ascontiguousarray(np.asarray(inputs["x"], dtype=np.float32))
    batch = np.asarray(inputs["batch"]).astype(np.int64)

    # ---- fold BN into the linears ----
    W1p, b1p = _fold_bn(
        np.asarray(inputs["W1"]), np.asarray(inputs["b1"]),
        np.asarray(inputs["g1"]), np.asarray(inputs["be1"]),
        np.asarray(inputs["m1"]), np.asarray(inputs["v1"]),
    )
    W2p, b2p = _fold_bn(
        np.asarray(inputs["W2"]), np.asarray(inputs["b2"]),
        np.asarray(inputs["g2"]), np.asarray(inputs["be2"]),
        np.asarray(inputs["m2"]), np.asarray(inputs["v2"]),
    )
    W3p, b3p = _fold_bn(
        np.asarray(inputs["W3"]), np.asarray(inputs["b3"]),
        np.asarray(inputs["g3"]), np.asarray(inputs["be3"]),
        np.asarray(inputs["m3"]), np.asarray(inputs["v3"]),
    )
    Wop, bop = _fold_bn(
        np.asarray(inputs["Wo"]), np.asarray(inputs["bo"]),
        np.asarray(inputs["go"]), np.asarray(inputs["beo"]),
        np.asarray(inputs["mo"]), np.asarray(inputs["vo"]),
    )

    # bf16 copies of the streaming-path weights (also used for the BIG bound)
    W1b = W1p.astype(BF16).astype(np.float32)
    W2b = W2p.astype(BF16).astype(np.float32)
    W3b = W3p.astype(BF16).astype(np.float32)

    # Pad columns are zero in x, so h2_pad is a known constant; BIG pushes the
    # padded layer-3 pre-activation strictly below zero (with margin for the
    # bf16 rounding of x/h tiles).
    h1_pad = np.maximum(b1p, 0.0)
    h2_pad = np.maximum(W2b @ h1_pad + b2p, 0.0)
    v3 = W3b @ h2_pad + b3p
    BIG = float(max(0.0, v3.max()) + 1000.0)

    # ---- whole-segment sharding by sorted-width round-robin rank ----
    counts = np.bincount(batch, minlength=NSEG).astype(np.int64)
    assert np.all(batch[:-1] <= batch[1:]), "batch must be sorted"
    order = np.argsort(-counts, kind="stable")  # segment ids, width desc
    slot_w = np.maximum(counts[order[::NCORES][:S]], 1)  # width of rank 8k
    tiles0, cols = _plan_tiles(slot_w)

    # per-tile tail-window width: the trailing columns of each slot that can
    # be padding on ANY core (only these need the -BIG mask matmul)
    wmat = counts[order[: S * NCORES]].reshape(S, NCORES)  # slot x core widths
    tiles = []
    for k0, d, wt, col0 in tiles0:
        minw = int(wmat[k0 : k0 + d].min())
        tailw = min(wt, (wt - minw + 1) & ~1)
        tiles.append((k0, d, wt, col0, tailw))

    key = (cols, float(BIG), tuple(slot_w.tolist()),
           tuple(t[4] for t in tiles), ACT_SLOTS, MAXENG, R2DVE)
    if key not in _compiled_cache:
        _compiled_cache[key] = _build_program(tiles, cols, BIG)
    nc = _compiled_cache[key]

    # column start and padded width of each slot
    slot_col = np.zeros(S, dtype=np.int64)
    slot_wt = np.zeros(S, dtype=np.int64)
    for k0, d, wt, col0, tailw in tiles:
        for j in range(d):
            slot_col[k0 + j] = col0 + j * wt
            slot_wt[k0 + j] = wt
    # epilogue shift corrections (DVE-shifted slots only): every window
    # column contributes -b3 of shift, pads included, so the correction is
    # +wt*b3 for sums and +b3 for maxes; ACT-fused slots need none.
    shifted = (np.arange(S) >= ACT_SLOTS).astype(np.float32)
    wtb3_host = np.ascontiguousarray(
        b3p[:, None] * (slot_wt.astype(np.float32) * shifted)[None, :]
    ).astype(np.float32)
    mb3_host = np.ascontiguousarray(
        b3p[:, None] * shifted[None, :]
    ).astype(np.float32)

    starts = np.searchsorted(batch, np.arange(NSEG), side="left")
    ends = np.searchsorted(batch, np.arange(NSEG), side="right")

    in_maps = []
    for c in range(NCORES):
        segs = order[np.arange(S) * NCORES + c]  # this core's segment ids
        src = np.full(cols, -1, dtype=np.int64)
        for k in range(S):
            s = segs[k]
            cnt = int(counts[s])
            if cnt:
                src[slot_col[k] : slot_col[k] + cnt] = np.arange(
                    starts[s], ends[s]
                )
        real = src >= 0
        xTc = np.zeros((C, cols), dtype=BF16)
        xTc[:, real] = x[src[real]].T.astype(BF16)
        auxc = np.zeros((1, cols), dtype=BF16)
        auxc[0, ~real] = 1.0
        recipc = (1.0 / np.maximum(counts[segs], 1.0)).astype(np.float32)
        in_maps.append(
            dict(
                xT=xTc,
                aux=auxc,
                w1=np.ascontiguousarray(W1p.T.astype(BF16)),
                w2=np.ascontiguousarray(W2p.T.astype(BF16)),
                w3=np.ascontiguousarray(W3p.T.astype(BF16)),
                b1=np.ascontiguousarray(b1p[:, None]),
                b2=np.ascontiguousarray(b2p[:, None]),
                b3=np.ascontiguousarray(b3p[:, None]),
                nbig=np.full((1, H), -BIG, BF16),
                wsum=np.ascontiguousarray(Wop[:, 0:H].T),
                wmax=np.ascontiguousarray(Wop[:, H : 2 * H].T),
                wmean=np.ascontiguousarray(Wop[:, 2 * H : 3 * H].T),
                bo=np.ascontiguousarray(bop[None, :]),
                recip=np.ascontiguousarray(recipc.reshape(S // H, H).T),
                nb3=np.ascontiguousarray(-b3p[:, None]),
                wtb3=wtb3_host,
                mb3=mb3_host,
            )
        )

    ncores_run = int(os.environ.get("KERNEL_NCORES", str(NCORES)))
    res = bass_utils.run_bass_kernel_spmd(
        nc,
        in_maps[:ncores_run],
        core_ids=list(range(ncores_run)),
        trace=bool(int(os.environ.get("KERNEL_TRACE", "0"))),
        tmpdir=os.environ.get("KERNEL_TRACE_DIR") or None,
    )
    kernel.last_results = res

    out_full = np.zeros((NSEG, O), dtype=np.float32)
    ranks = np.arange(S)
    for c in range(ncores_run):
        out_full[order[ranks * NCORES + c]] = res.results[c]["out"]
    return out_full
